# revision 33
# baseline (speedup 1.0000x reference)
"""Trainium2 Bass kernel for nn_BatchInfoNCELoss.

Reference semantics: unfold 3x3 patches of latents [B=9,H=768,W=768,C=3],
L2-normalize, pick ~100 anchor positions + their 13-offset neighborhoods,
compute cross-image squared cosine similarities and a masked weighted mean.

Key algebraic observation: the loss only consumes the normalized patches at
the ~100 anchor positions and their 13 neighbor positions (per image), i.e.
9*100*13*27 floats.  All index math, the tiny gather and the normalization
are host-side prep; the O(B^2 * n * M * D) similarity reduction runs on the
8 NeuronCores, sharded over the anchor axis (13 anchor slots per core).

Per core the device computes, with all loss weights folded into the
operands on the host:
    U[p, f] = sum_d lhsT[d, p] * rhs[d, f]      (one 27-contract matmul)
    out[p]  = sum_f (U * mask) * U              (masked square-sum)
where p = (anchor_slot, b) [117 used of 128] and f = (anchor_slot', j, m)
[1521 used of 1536, split in 4 chunks of 384 to fit PSUM banks].
mask = [slot==slot'] * [j != b] (bf16 0/1).  The host sums the 8 per-core
[128,4] partial accumulators into the scalar loss.
"""

import sys

sys.path.insert(0, "/opt/trn_rl_repo")

import numpy as np
import ml_dtypes

def _ensure_axon_hooks():
    """The container's antenv stub lacks axon_hooks; provide it so the axon
    boot can register its NTFF profile hook and bass_utils can read it when
    tracing is requested (BASS_TRACE=1). No-op if the real module exists."""
    try:
        import antenv.axon_hooks  # noqa: F401
        return
    except ImportError:
        pass
    import types

    import antenv

    mod = types.ModuleType("antenv.axon_hooks")
    mod._hook = None

    def set_axon_ntff_profile_hook(hook):
        mod._hook = hook

    def get_axon_ntff_profile_hook():
        return mod._hook

    mod.set_axon_ntff_profile_hook = set_axon_ntff_profile_hook
    mod.get_axon_ntff_profile_hook = get_axon_ntff_profile_hook
    sys.modules["antenv.axon_hooks"] = mod
    antenv.axon_hooks = mod


_ensure_axon_hooks()

import concourse.bass as bass
import concourse.tile as tile
from concourse import mybir
from concourse.bass_utils import run_bass_kernel_spmd
from concourse.vector_clock import ScopedClock, VectorClock


def _split_drain_and_barrier(self, tick_clock, wait_clock):
    """Replacement for TileContext._drain_and_barrier that emits one drain
    per outstanding semaphore: this walrus build rejects instructions that
    carry more than one sync wait ("Too many sync wait commands")."""
    ticks = list(tick_clock.global_clock)
    for proc, tick in enumerate(ticks):
        if tick == 0:
            continue
        partial = [0] * len(ticks)
        partial[proc] = tick
        drain_inst = self.nc.sync.drain()
        wait_clock.add_sem_waits(
            drain_inst.ins, ScopedClock({None: VectorClock(partial)})
        )
    self.nc.all_engine_barrier()
    assert self.sems is not None
    popped = self.nc._tile_sem_poison_stack.pop()
    assert popped is self._sem_poison
    self.nc.clear_and_free_semaphores(list(self.sems.allocated().values()))
    self.nc.all_engine_barrier()


tile.TileContext._drain_and_barrier = _split_drain_and_barrier

# ---- problem constants (hardcoded per contract) ----
B, H, W, C = 9, 768, 768, 3
PATCH = 3
TEMPERATURE = 0.5
RADIUS = 2.0
NS = 100          # number of anchors
EPS = 1e-12
D = PATCH * PATCH * C          # 27
_r = int(np.floor(RADIUS))
OFFSETS = np.array(
    [(dy, dx) for dy in range(-_r, _r + 1) for dx in range(-_r, _r + 1)
     if dy * dy + dx * dx <= RADIUS * RADIUS],
    dtype=np.int64,
)
M = len(OFFSETS)               # 13
CENTER = 6                     # index of offset (0,0) in OFFSETS

N_CORES = 8
NL = 13                        # anchor slots per core (8*13 = 104 >= 100)
NSLOT = 16                     # padded slots per core: 4 groups of 4
N_GROUP = 4                    # block-diag groups; contract K = 4*27 = 108
KC = N_GROUP * D               # 108
GS = N_GROUP * B               # 36 = output rows per group (slot, b)
GW = N_GROUP * B * M           # 468 = free width per group
GCOLS = GS + GW                # 504: per-group packed cols (lhsT | rhs)

LAST_RESULTS = None            # BassKernelResults of the most recent run


def _build_nc():
    f32 = mybir.dt.float32
    f16 = mybir.dt.float16
    nc = bass.Bass()
    # Groups packed contiguously per partition: one DMA of 108 rows x 4032B
    # (the DMA engines are descriptor-paced, so few big rows beat many small
    # ones).  Per group g: cols [0, 36) = lhsT_g (block-diag anchors), cols
    # [36, 504) = rhs_g (block-diag neighborhoods).  fp16 operands: one PE
    # pass per matmul (fp32 takes two) and half the DMA bytes; PSUM
    # accumulation stays fp32, and the quadratic sum averages the rounding
    # noise out (measured 7e-7 rel err offline).  Block-diagonal contract
    # slices make cross-anchor products exactly 0; the j==b diagonal is
    # subtracted on the host.
    in_d = nc.dram_tensor("in_pack", [KC, N_GROUP, GCOLS], f16, kind="ExternalInput")
    out_d = nc.dram_tensor("acc_out", [GS, N_GROUP], f32, kind="ExternalOutput")

    KH = KC // 2
    with tile.TileContext(nc) as tc:
        with (
            tc.tile_pool(name="sb", bufs=1) as sb,
            tc.tile_pool(name="work", bufs=6) as work,
            tc.tile_pool(name="ps", bufs=1, space="PSUM") as ps,
        ):
            inp = sb.tile([KC, N_GROUP, GCOLS], f16)
            # Two half-DMAs on the two HWDGE-capable queues (sync + scalar)
            # so more DMA engines run in parallel.
            nc.sync.dma_start(out=inp[:KH], in_=in_d[:KH])
            nc.scalar.dma_start(out=inp[KH:], in_=in_d[KH:])
            # 1x1 dummy matmul absorbs the first half's DMA wait on the PE:
            # the first real matmul then carries only the second half's wait
            # (one sync wait per compute instruction).
            dummy = ps.tile([1, 1], f32, bufs=1)
            nc.tensor.matmul(dummy, inp[:1, 0, :1], inp[:1, 0, :1],
                             start=True, stop=True)
            acc_a = work.tile([GS, 3], f32)   # ACT partials (groups 0-2)
            acc_b = work.tile([GS, 1], f32)   # DVE partial (group 3)
            for g in range(N_GROUP):
                u = ps.tile([GS, GW], f32, tag="u", bufs=4)
                nc.tensor.matmul(u, inp[:, g, :GS], inp[:, g, GS:],
                                 start=True, stop=True)
                if g < 3:
                    sq = work.tile([GS, GW], f32, tag="sq")
                    nc.scalar.activation(out=sq, in_=u,
                                         func=mybir.ActivationFunctionType.Square,
                                         accum_out=acc_a[:, g:g + 1])
                else:
                    # DVE path (parallel with ACT); DVE may read only one
                    # operand from PSUM, so stage a copy.
                    uc = work.tile([GS, GW], f32, tag="uc")
                    nc.vector.tensor_copy(uc, u)
                    sq2 = work.tile([GS, GW], f32, tag="sq2")
                    nc.vector.tensor_mul(sq2, uc, u)
                    nc.vector.reduce_sum(acc_b[:, 0:1], sq2,
                                         axis=mybir.AxisListType.X)
            # Separate accumulators per engine -> each output DMA carries a
            # single cross-engine wait.
            nc.sync.dma_start(out=out_d[:, :3], in_=acc_a)
            nc.scalar.dma_start(out=out_d[:, 3:], in_=acc_b)
    return nc


def _host_prep(latents, anchor_indices):
    """Gather + normalize + weight-fold; returns per-core device inputs."""
    lat = np.ascontiguousarray(np.asarray(latents), dtype=np.float32)
    ai = np.asarray(anchor_indices).astype(np.int64)

    ay, ax = ai // W, ai % W
    ny = ay[:, None] + OFFSETS[None, :, 0]
    nx = ax[:, None] + OFFSETS[None, :, 1]
    valid = (ny >= 0) & (ny < H) & (nx >= 0) & (nx < W)          # [NS, M]
    pos = np.clip(ny, 0, H - 1) * W + np.clip(nx, 0, W - 1)      # [NS, M]
    counts = valid.sum(1).astype(np.float32)                     # [NS]

    # 3x3 patch pixel indices (edge-clamped) for every needed position
    pf = pos.reshape(-1)
    py, px = pf // W, pf % W
    d3 = np.arange(PATCH) - PATCH // 2
    yy = np.clip(py[:, None, None] + d3[None, :, None], 0, H - 1)
    xx = np.clip(px[:, None, None] + d3[None, None, :], 0, W - 1)
    lin = (yy * W + xx).reshape(-1, PATCH * PATCH)               # [NS*M, 9]
    g = lat.reshape(B, H * W, C)[:, lin, :].reshape(B, NS, M, D)
    nrm = np.sqrt((g * g).sum(-1, keepdims=True))
    gn = g / np.maximum(nrm, np.float32(EPS))                    # [B, NS, M, D]

    K = B - 1
    c1 = np.float32(1.0 / (TEMPERATURE * np.sqrt(K * B * NS)))
    w2 = np.sqrt(valid.astype(np.float32) / counts[:, None])     # [NS, M]
    A = gn[:, :, CENTER, :] * c1                                 # [B, NS, D]
    N = gn * w2[None, :, :, None]                                # [B, NS, M, D]

    # j==b diagonal correction, subtracted on the host (f64 accumulation)
    diag = np.einsum("bnd,bnmd->bnm", A.astype(np.float64), N.astype(np.float64))
    diag_sum = float((diag * diag).sum())

    # Per-core packed input [108, 4, 504]: per group, lhsT_g | rhs_g,
    # both block-diagonal over the group's 4 anchor slots.
    packs = np.zeros((N_CORES, N_GROUP, KC, GCOLS), np.float32)
    for c in range(N_CORES):
        n0 = c * NL
        ns = max(0, min(NL, NS - n0))
        Ac = np.zeros((NSLOT, B, D), np.float32)
        Nc = np.zeros((NSLOT, B, M, D), np.float32)
        Ac[:ns] = A[:, n0:n0 + ns].transpose(1, 0, 2)
        Nc[:ns] = N[:, n0:n0 + ns].transpose(1, 0, 2, 3)
        for g in range(N_GROUP):
            pack = packs[c, g]
            for sl in range(N_GROUP):
                s = N_GROUP * g + sl
                if s >= NL:
                    continue
                rows = slice(sl * D, (sl + 1) * D)
                pack[rows, sl * B:(sl + 1) * B] = Ac[s].T        # [D, B]
                rc = GS + sl * B * M
                pack[rows, rc:rc + B * M] = Nc[s].reshape(B * M, D).T
    packs16 = np.ascontiguousarray(
        packs.transpose(0, 2, 1, 3)  # [core, KC, group, GCOLS]
    ).astype(np.float16)
    return packs16, diag_sum


def kernel(latents, anchor_indices):
    global LAST_RESULTS
    # Initialize jax first: the axon boot registers the NTFF profile hook at
    # platform init, and run_bass_kernel_spmd checks the hook before running.
    import jax

    jax.devices()
    packs, diag_sum = _host_prep(latents, anchor_indices)
    nc = _build_nc()
    in_maps = [{"in_pack": packs[c]} for c in range(N_CORES)]
    res = run_bass_kernel_spmd(nc, in_maps, core_ids=list(range(N_CORES)))
    LAST_RESULTS = res
    total = np.float64(0.0)
    for r in res.results:
        total += r["acc_out"].astype(np.float64).sum()
    return np.float32(total - diag_sum)


# revision 34
# speedup vs baseline: 1.1122x; 1.1122x over previous
"""Trainium2 Bass kernel for nn_BatchInfoNCELoss.

Reference semantics: unfold 3x3 patches of latents [B=9,H=768,W=768,C=3],
L2-normalize, pick ~100 anchor positions + their 13-offset neighborhoods,
compute cross-image squared cosine similarities and a masked weighted mean.

Key algebraic observation: the loss only consumes the normalized patches at
the ~100 anchor positions and their 13 neighbor positions (per image), i.e.
9*100*13*27 floats.  All index math, the tiny gather and the normalization
are host-side prep; the O(B^2 * n * M * D) similarity reduction runs on the
8 NeuronCores, sharded over the anchor axis (13 anchor slots per core).

Per core the device computes, with all loss weights folded into the
operands on the host:
    U[p, f] = sum_d lhsT[d, p] * rhs[d, f]      (one 27-contract matmul)
    out[p]  = sum_f (U * mask) * U              (masked square-sum)
where p = (anchor_slot, b) [117 used of 128] and f = (anchor_slot', j, m)
[1521 used of 1536, split in 4 chunks of 384 to fit PSUM banks].
mask = [slot==slot'] * [j != b] (bf16 0/1).  The host sums the 8 per-core
[128,4] partial accumulators into the scalar loss.
"""

import sys

sys.path.insert(0, "/opt/trn_rl_repo")

import numpy as np
import ml_dtypes

def _ensure_axon_hooks():
    """The container's antenv stub lacks axon_hooks; provide it so the axon
    boot can register its NTFF profile hook and bass_utils can read it when
    tracing is requested (BASS_TRACE=1). No-op if the real module exists."""
    try:
        import antenv.axon_hooks  # noqa: F401
        return
    except ImportError:
        pass
    import types

    import antenv

    mod = types.ModuleType("antenv.axon_hooks")
    mod._hook = None

    def set_axon_ntff_profile_hook(hook):
        mod._hook = hook

    def get_axon_ntff_profile_hook():
        return mod._hook

    mod.set_axon_ntff_profile_hook = set_axon_ntff_profile_hook
    mod.get_axon_ntff_profile_hook = get_axon_ntff_profile_hook
    sys.modules["antenv.axon_hooks"] = mod
    antenv.axon_hooks = mod


_ensure_axon_hooks()

import concourse.bass as bass
import concourse.tile as tile
from concourse import mybir
from concourse.bass_utils import run_bass_kernel_spmd
from concourse.vector_clock import ScopedClock, VectorClock


def _split_drain_and_barrier(self, tick_clock, wait_clock):
    """Replacement for TileContext._drain_and_barrier that emits one drain
    per outstanding semaphore: this walrus build rejects instructions that
    carry more than one sync wait ("Too many sync wait commands")."""
    ticks = list(tick_clock.global_clock)
    for proc, tick in enumerate(ticks):
        if tick == 0:
            continue
        partial = [0] * len(ticks)
        partial[proc] = tick
        drain_inst = self.nc.sync.drain()
        wait_clock.add_sem_waits(
            drain_inst.ins, ScopedClock({None: VectorClock(partial)})
        )
    self.nc.all_engine_barrier()
    assert self.sems is not None
    popped = self.nc._tile_sem_poison_stack.pop()
    assert popped is self._sem_poison
    self.nc.clear_and_free_semaphores(list(self.sems.allocated().values()))
    self.nc.all_engine_barrier()


tile.TileContext._drain_and_barrier = _split_drain_and_barrier

# ---- problem constants (hardcoded per contract) ----
B, H, W, C = 9, 768, 768, 3
PATCH = 3
TEMPERATURE = 0.5
RADIUS = 2.0
NS = 100          # number of anchors
EPS = 1e-12
D = PATCH * PATCH * C          # 27
_r = int(np.floor(RADIUS))
OFFSETS = np.array(
    [(dy, dx) for dy in range(-_r, _r + 1) for dx in range(-_r, _r + 1)
     if dy * dy + dx * dx <= RADIUS * RADIUS],
    dtype=np.int64,
)
M = len(OFFSETS)               # 13
CENTER = 6                     # index of offset (0,0) in OFFSETS

N_CORES = 8
NL = 13                        # anchor slots per core (8*13 = 104 >= 100)
NSLOT = 16                     # padded slots per core: 4 groups of 4
N_GROUP = 4                    # block-diag groups; contract K = 4*27 = 108
KC = N_GROUP * D               # 108
GS = N_GROUP * B               # 36 = output rows per group (slot, b)
GW = N_GROUP * B * M           # 468 = free width per group
GCOLS = GS + GW                # 504: per-group packed cols (lhsT | rhs)

LAST_RESULTS = None            # BassKernelResults of the most recent run


def _build_nc():
    f32 = mybir.dt.float32
    f16 = mybir.dt.float16
    nc = bass.Bass()
    # Groups packed contiguously per partition: one DMA of 108 rows x 4032B
    # (the DMA engines are descriptor-paced, so few big rows beat many small
    # ones).  Per group g: cols [0, 36) = lhsT_g (block-diag anchors), cols
    # [36, 504) = rhs_g (block-diag neighborhoods).  fp16 operands: one PE
    # pass per matmul (fp32 takes two) and half the DMA bytes; PSUM
    # accumulation stays fp32, and the quadratic sum averages the rounding
    # noise out (measured 7e-7 rel err offline).  Block-diagonal contract
    # slices make cross-anchor products exactly 0; the j==b diagonal is
    # subtracted on the host.
    in_d = nc.dram_tensor("in_pack", [KC, N_GROUP, GCOLS], f16, kind="ExternalInput")
    out_d = nc.dram_tensor("acc_out", [GS, N_GROUP], f32, kind="ExternalOutput")

    with tile.TileContext(nc) as tc:
        with (
            tc.tile_pool(name="sb", bufs=1) as sb,
            tc.tile_pool(name="work", bufs=4) as work,
            tc.tile_pool(name="ps", bufs=4, space="PSUM") as ps,
        ):
            inp = sb.tile([KC, N_GROUP, GCOLS], f16)
            nc.sync.dma_start(out=inp, in_=in_d[:])
            acc = work.tile([GS, N_GROUP], f32)
            for g in range(N_GROUP):
                u = ps.tile([GS, GW], f32, tag="u")
                nc.tensor.matmul(u, inp[:, g, :GS], inp[:, g, GS:],
                                 start=True, stop=True)
                sq = work.tile([GS, GW], f32, tag="sq")
                nc.scalar.activation(out=sq, in_=u,
                                     func=mybir.ActivationFunctionType.Square,
                                     accum_out=acc[:, g:g + 1])
            nc.sync.dma_start(out=out_d[:], in_=acc)
    return nc


def _host_prep(latents, anchor_indices):
    """Gather + normalize + weight-fold; returns per-core device inputs."""
    lat = np.ascontiguousarray(np.asarray(latents), dtype=np.float32)
    ai = np.asarray(anchor_indices).astype(np.int64)

    ay, ax = ai // W, ai % W
    ny = ay[:, None] + OFFSETS[None, :, 0]
    nx = ax[:, None] + OFFSETS[None, :, 1]
    valid = (ny >= 0) & (ny < H) & (nx >= 0) & (nx < W)          # [NS, M]
    pos = np.clip(ny, 0, H - 1) * W + np.clip(nx, 0, W - 1)      # [NS, M]
    counts = valid.sum(1).astype(np.float32)                     # [NS]

    # 3x3 patch pixel indices (edge-clamped) for every needed position
    pf = pos.reshape(-1)
    py, px = pf // W, pf % W
    d3 = np.arange(PATCH) - PATCH // 2
    yy = np.clip(py[:, None, None] + d3[None, :, None], 0, H - 1)
    xx = np.clip(px[:, None, None] + d3[None, None, :], 0, W - 1)
    lin = (yy * W + xx).reshape(-1, PATCH * PATCH)               # [NS*M, 9]
    g = lat.reshape(B, H * W, C)[:, lin, :].reshape(B, NS, M, D)
    nrm = np.sqrt((g * g).sum(-1, keepdims=True))
    gn = g / np.maximum(nrm, np.float32(EPS))                    # [B, NS, M, D]

    K = B - 1
    c1 = np.float32(1.0 / (TEMPERATURE * np.sqrt(K * B * NS)))
    w2 = np.sqrt(valid.astype(np.float32) / counts[:, None])     # [NS, M]
    A = gn[:, :, CENTER, :] * c1                                 # [B, NS, D]
    N = gn * w2[None, :, :, None]                                # [B, NS, M, D]

    # j==b diagonal correction, subtracted on the host (f64 accumulation)
    diag = np.einsum("bnd,bnmd->bnm", A.astype(np.float64), N.astype(np.float64))
    diag_sum = float((diag * diag).sum())

    # Per-core packed input [108, 4, 504]: per group, lhsT_g | rhs_g,
    # both block-diagonal over the group's 4 anchor slots.
    packs = np.zeros((N_CORES, N_GROUP, KC, GCOLS), np.float32)
    for c in range(N_CORES):
        n0 = c * NL
        ns = max(0, min(NL, NS - n0))
        Ac = np.zeros((NSLOT, B, D), np.float32)
        Nc = np.zeros((NSLOT, B, M, D), np.float32)
        Ac[:ns] = A[:, n0:n0 + ns].transpose(1, 0, 2)
        Nc[:ns] = N[:, n0:n0 + ns].transpose(1, 0, 2, 3)
        for g in range(N_GROUP):
            pack = packs[c, g]
            for sl in range(N_GROUP):
                s = N_GROUP * g + sl
                if s >= NL:
                    continue
                rows = slice(sl * D, (sl + 1) * D)
                pack[rows, sl * B:(sl + 1) * B] = Ac[s].T        # [D, B]
                rc = GS + sl * B * M
                pack[rows, rc:rc + B * M] = Nc[s].reshape(B * M, D).T
    packs16 = np.ascontiguousarray(
        packs.transpose(0, 2, 1, 3)  # [core, KC, group, GCOLS]
    ).astype(np.float16)
    return packs16, diag_sum


def kernel(latents, anchor_indices):
    global LAST_RESULTS
    # Initialize jax first: the axon boot registers the NTFF profile hook at
    # platform init, and run_bass_kernel_spmd checks the hook before running.
    import jax

    jax.devices()
    packs, diag_sum = _host_prep(latents, anchor_indices)
    nc = _build_nc()
    in_maps = [{"in_pack": packs[c]} for c in range(N_CORES)]
    res = run_bass_kernel_spmd(nc, in_maps, core_ids=list(range(N_CORES)))
    LAST_RESULTS = res
    total = np.float64(0.0)
    for r in res.results:
        total += r["acc_out"].astype(np.float64).sum()
    return np.float32(total - diag_sum)
